# revision 4
# baseline (speedup 1.0000x reference)
"""ChannelAttention (XCA-style cross-covariance attention) TRN2 kernel.

Shapes (hardcoded): x [8, 128, 128, 128] f32 (B, H, W, C), C=128, heads=4,
hd=32, N = H*W = 16384 tokens per sample. 8 NeuronCores, data-parallel over
batch: core i processes sample i, weights replicated, no collectives.

Algebraic reduction: attention is over channels with l2-normalization over
the full token axis, so per sample everything collapses to
  S   = [X|1]^T-style Gram stats:  S = X^T X (128x128), s = X^T 1 (128)
  G   = Wq^T S Wk + q_bias (x) (s^T Wk)          (per-head 32x32 diag blocks)
  sqq = diag(Wq^T S Wq) + 2 qb*(s^T Wq) + N qb^2 ; sqk = diag(Wk^T S Wk)
  logits_h = exp(scale_h) * rsqrt(sqq) * G * rsqrt(sqk) ; A = softmax rows
  P   = blockdiag(A)^T @ proj_w ;  Wf = Wv @ P ;  bf = v_bias @ P + proj_b
  Y   = X @ Wf + bf
Two streaming passes over X (Gram+transpose, then the output GEMM) plus a
tiny serial middle section.
"""

import os
import sys
import types

import numpy as np

from concourse import bacc, mybir
import concourse.bass as bass
import concourse.tile as tile
from concourse.bass_utils import run_bass_kernel_spmd
from concourse.masks import make_identity

F32 = mybir.dt.float32
BF16 = mybir.dt.bfloat16

B, H, W, C = 8, 128, 128, 128
NTOK = H * W          # 16384 tokens per sample
NT = NTOK // 128      # 128 token-tiles of 128 tokens
CHUNK = 8             # token-tiles per DMA chunk
NCH = NT // CHUNK     # 16 chunks
HEADS, HD = 4, 32
EPS = 1.55e-05

LAST_EXEC_TIME_NS = None
_CACHED_NC = None


def _install_ntff_hook():
    """Register the axon NTFF profile hook if the image's antenv lacks it."""
    try:
        import antenv.axon_hooks  # noqa: F401
        return
    except ImportError:
        pass
    try:
        from trn_agent_boot.trn_boot import _ntff_profile_via_ctypes
        hook = _ntff_profile_via_ctypes("/opt/axon/libaxon_pjrt.so")
        mod = types.ModuleType("antenv.axon_hooks")
        mod.get_axon_ntff_profile_hook = lambda: hook
        sys.modules["antenv.axon_hooks"] = mod
    except Exception:
        pass


def build():
    nc = bacc.Bacc(None, target_bir_lowering=False)

    x_d = nc.declare_dram_parameter("x", [NTOK, C], F32, isOutput=False)
    qkvw_d = nc.declare_dram_parameter("qkv_w", [C, 3 * C], F32, isOutput=False)
    qb_d = nc.declare_dram_parameter("q_bias", [C], F32, isOutput=False)
    vb_d = nc.declare_dram_parameter("v_bias", [C], F32, isOutput=False)
    kb_d = nc.declare_dram_parameter("k_bias", [C], F32, isOutput=False)
    sc_d = nc.declare_dram_parameter("scale", [HEADS], F32, isOutput=False)
    pw_d = nc.declare_dram_parameter("proj_w", [C, C], F32, isOutput=False)
    pb_d = nc.declare_dram_parameter("proj_b", [C], F32, isOutput=False)
    out_d = nc.declare_dram_parameter("out", [NTOK, C], F32, isOutput=True)

    x_t = x_d.ap().rearrange("(ch n p) c -> ch p n c", p=128, n=CHUNK)
    out_t = out_d.ap().rearrange("(ch n p) c -> ch p n c", p=128, n=CHUNK)

    from contextlib import ExitStack

    with tile.TileContext(nc) as tc:
        with (
            tc.tile_pool(name="singles", bufs=1) as singles,
            tc.tile_pool(name="mid", bufs=1) as mid,
        ):
            mid_ctx = ExitStack()
            psum_s = mid_ctx.enter_context(
                tc.tile_pool(name="psum_s", bufs=1, space="PSUM"))
            psum_mid = mid_ctx.enter_context(
                tc.tile_pool(name="psum_mid", bufs=4, space="PSUM"))
            # ---- constants / weights -------------------------------------
            ident_bf = singles.tile([128, 128], BF16)
            make_identity(nc, ident_bf[:])
            ident_f32 = singles.tile([128, 128], F32)
            make_identity(nc, ident_f32[:])

            w_sb = singles.tile([C, 3 * C], F32)
            nc.sync.dma_start(w_sb[:], qkvw_d[:, :])
            pw_sb = singles.tile([C, C], F32)
            nc.sync.dma_start(pw_sb[:], pw_d[:, :])
            qb_row = singles.tile([1, C], F32)
            nc.sync.dma_start(qb_row[:], qb_d[None, :])
            vb_row = singles.tile([1, C], F32)
            nc.sync.dma_start(vb_row[:], vb_d[None, :])
            kb_row = singles.tile([1, C], F32)
            nc.sync.dma_start(kb_row[:], kb_d[None, :])
            pb_row = singles.tile([1, C], F32)
            nc.sync.dma_start(pb_row[:], pb_d[None, :])
            sc_row = singles.tile([1, HEADS], F32)
            nc.sync.dma_start(sc_row[:], sc_d[None, :])

            one_one = singles.tile([1, 1], F32)
            nc.vector.memset(one_one[:], 1.0)
            ones_row = singles.tile([1, C], F32)
            nc.vector.memset(ones_row[:], 1.0)
            ones_col = singles.tile([128, 1], F32)
            nc.vector.memset(ones_col[:], 1.0)

            # exp(scale) broadcast to [1, 128] (32 copies per head)
            esc4 = singles.tile([1, HEADS], F32)
            nc.scalar.activation(esc4[:], sc_row[:], mybir.ActivationFunctionType.Exp)
            esc_row = singles.tile([1, C], F32)
            nc.vector.tensor_copy(
                esc_row[:].rearrange("p (a b) -> p a b", b=HD),
                esc4[:, :, None].to_broadcast((1, HEADS, HD)),
            )

            # x-independent middle pieces: Wv^T and v_bias as a column
            wvT_sb = mid.tile([C, C], F32)
            wvT_ps = psum_mid.tile([C, C], F32, tag="mps")
            nc.tensor.matmul(wvT_ps[:], lhsT=w_sb[:, 2 * C:3 * C], rhs=ident_f32[:],
                             start=True, stop=True)
            nc.vector.tensor_copy(wvT_sb[:], wvT_ps[:])

            vb_col = mid.tile([C, 1], F32)
            vbc_ps = psum_mid.tile([C, 1], F32, tag="mps")
            nc.tensor.matmul(vbc_ps[:], lhsT=vb_row[:], rhs=one_one[:],
                             start=True, stop=True)
            nc.vector.tensor_copy(vb_col[:], vbc_ps[:])

            # ---- pass 1: Gram stats + transpose of x ---------------------
            xT_store = singles.tile([C, NTOK], BF16)
            s_ps = psum_s.tile([C, C + 1], F32)

            with (
                tc.tile_pool(name="xin", bufs=3) as xin_pool,
                tc.tile_pool(name="xbf", bufs=3) as xbf_pool,
                tc.tile_pool(name="psum_xt", bufs=3, space="PSUM") as psum_xt,
            ):
                for ch in range(NCH):
                    xin = xin_pool.tile([128, CHUNK, C], F32)
                    nc.sync.dma_start(xin[:], x_t[ch])
                    for n in range(CHUNK):
                        g = ch * CHUNK + n
                        xb = xbf_pool.tile([128, C + 1], BF16)
                        nc.vector.tensor_copy(xb[:, 0:C], xin[:, n, :])
                        nc.vector.memset(xb[:, C:C + 1], 1.0)
                        nc.tensor.matmul(s_ps[:], lhsT=xb[:, 0:C], rhs=xb[:],
                                         start=(g == 0), stop=(g == NT - 1))
                        xt_ps = psum_xt.tile([C, 128], F32)
                        nc.tensor.matmul(xt_ps[:], lhsT=xb[:, 0:C], rhs=ident_bf[:],
                                         start=True, stop=True)
                        nc.scalar.copy(xT_store[:, g * 128:(g + 1) * 128], xt_ps[:])

            # ---- middle: attention matrix -> Wf, bf ----------------------
            s_sb = mid.tile([C, C + 1], F32)
            nc.vector.tensor_copy(s_sb[:], s_ps[:])

            # SW = S @ [Wq | Wk]  (S symmetric)
            sw_ps = psum_mid.tile([C, 2 * C], F32, tag="mps")
            nc.tensor.matmul(sw_ps[:], lhsT=s_sb[:, 0:C], rhs=w_sb[:, 0:2 * C],
                             start=True, stop=True)
            sw_sb = mid.tile([C, 2 * C], F32)
            nc.vector.tensor_copy(sw_sb[:], sw_ps[:])

            # srow = s^T [Wq | Wk]
            srow_ps = psum_mid.tile([1, 2 * C], F32, tag="mps")
            nc.tensor.matmul(srow_ps[:], lhsT=s_sb[:, C:C + 1], rhs=w_sb[:, 0:2 * C],
                             start=True, stop=True)
            srow_sb = mid.tile([1, 2 * C], F32)
            nc.vector.tensor_copy(srow_sb[:], srow_ps[:])

            # sq = colsum([Wq|Wk] .* SW) = [diag(Wq^T S Wq) | diag(Wk^T S Wk)]
            prod_sb = mid.tile([C, 2 * C], F32)
            nc.vector.tensor_mul(prod_sb[:], w_sb[:, 0:2 * C], sw_sb[:])
            sq_ps = psum_mid.tile([1, 2 * C], F32, tag="mps")
            nc.tensor.matmul(sq_ps[:], lhsT=ones_col[:], rhs=prod_sb[:],
                             start=True, stop=True)
            sq_sb = mid.tile([1, 2 * C], F32)
            nc.vector.tensor_copy(sq_sb[:], sq_ps[:])

            # G = Wq^T S Wk + qb (x) (srow_k + N*kb) + (Wq^T s) (x) kb
            srowkn = mid.tile([1, C], F32)
            nc.vector.tensor_scalar_mul(srowkn[:], kb_row[:], float(NTOK))
            nc.vector.tensor_add(srowkn[:], srowkn[:], srow_sb[:, C:2 * C])
            g_ps = psum_mid.tile([C, C], F32, tag="mps")
            nc.tensor.matmul(g_ps[:], lhsT=w_sb[:, 0:C], rhs=sw_sb[:, C:2 * C],
                             start=True, stop=False)
            nc.tensor.matmul(g_ps[:], lhsT=qb_row[:], rhs=srowkn[:],
                             start=False, stop=False)
            nc.tensor.matmul(g_ps[:], lhsT=srow_sb[:, 0:C], rhs=kb_row[:],
                             start=False, stop=True)

            # sq_q += 2*qb*srow_q + NTOK*qb^2 ; then rq/rk = rsqrt(max(sq, EPS))
            sqq_row = mid.tile([1, C], F32)
            t_a = mid.tile([1, C], F32)
            nc.vector.tensor_mul(t_a[:], qb_row[:], qb_row[:])
            nc.vector.tensor_scalar_mul(t_a[:], t_a[:], float(NTOK))
            t_b = mid.tile([1, C], F32)
            nc.vector.tensor_mul(t_b[:], qb_row[:], srow_sb[:, 0:C])
            nc.vector.tensor_scalar_mul(t_b[:], t_b[:], 2.0)
            nc.vector.tensor_add(sqq_row[:], sq_sb[:, 0:C], t_a[:])
            nc.vector.tensor_add(sqq_row[:], sqq_row[:], t_b[:])

            rq_row = mid.tile([1, C], F32)
            nc.vector.tensor_scalar_max(rq_row[:], sqq_row[:], EPS)
            nc.scalar.sqrt(rq_row[:], rq_row[:])
            nc.vector.reciprocal(rq_row[:], rq_row[:])
            nc.vector.tensor_mul(rq_row[:], rq_row[:], esc_row[:])

            sqk_row = mid.tile([1, C], F32)
            t_c = mid.tile([1, C], F32)
            nc.vector.tensor_mul(t_c[:], kb_row[:], kb_row[:])
            nc.vector.tensor_scalar_mul(t_c[:], t_c[:], float(NTOK))
            t_d = mid.tile([1, C], F32)
            nc.vector.tensor_mul(t_d[:], kb_row[:], srow_sb[:, C:2 * C])
            nc.vector.tensor_scalar_mul(t_d[:], t_d[:], 2.0)
            nc.vector.tensor_add(sqk_row[:], sq_sb[:, C:2 * C], t_c[:])
            nc.vector.tensor_add(sqk_row[:], sqk_row[:], t_d[:])

            rk_row = mid.tile([1, C], F32)
            nc.vector.tensor_scalar_max(rk_row[:], sqk_row[:], EPS)
            nc.scalar.sqrt(rk_row[:], rk_row[:])
            nc.vector.reciprocal(rk_row[:], rk_row[:])

            # rq as a column (includes exp(scale)); rk broadcast to all rows
            rqc_ps = psum_mid.tile([C, 1], F32, tag="mps")
            nc.tensor.matmul(rqc_ps[:], lhsT=rq_row[:], rhs=one_one[:],
                             start=True, stop=True)
            rq_col = mid.tile([C, 1], F32)
            nc.vector.tensor_copy(rq_col[:], rqc_ps[:])

            rkb_ps = psum_mid.tile([C, C], F32, tag="mps")
            nc.tensor.matmul(rkb_ps[:], lhsT=ones_row[:], rhs=rk_row[:],
                             start=True, stop=True)
            rk_bc = mid.tile([C, C], F32)
            nc.vector.tensor_copy(rk_bc[:], rkb_ps[:])

            # per-head 32x32 logits blocks; softmax over rows
            blk = mid.tile([128, HD], F32)
            for h in range(HEADS):
                r = slice(h * HD, (h + 1) * HD)
                nc.vector.tensor_scalar(blk[r, :], g_ps[r, r], rq_col[r, 0:1], None,
                                        op0=mybir.AluOpType.mult)
                nc.vector.tensor_mul(blk[r, :], blk[r, :], rk_bc[r, r])

            mx = mid.tile([128, 1], F32)
            nc.vector.reduce_max(mx[:], blk[:], axis=mybir.AxisListType.X)
            nc.vector.tensor_scalar(blk[:], blk[:], mx[:, 0:1], None,
                                    op0=mybir.AluOpType.subtract)
            sumx = mid.tile([128, 1], F32)
            nc.scalar.activation(blk[:], blk[:], mybir.ActivationFunctionType.Exp,
                                 accum_out=sumx[:])
            rs = mid.tile([128, 1], F32)
            nc.vector.reciprocal(rs[:], sumx[:])

            attn_big = mid.tile([128, 128], F32)
            nc.vector.memset(attn_big[:], 0.0)
            for h in range(HEADS):
                r = slice(h * HD, (h + 1) * HD)
                nc.vector.tensor_scalar(attn_big[r, r], blk[r, :], rs[r, 0:1], None,
                                        op0=mybir.AluOpType.mult)

            # P = blockdiag(A)^T @ proj_w ; Wf = Wv @ P ; bf = vb @ P + pb
            p_ps = psum_mid.tile([C, C], F32, tag="mps")
            nc.tensor.matmul(p_ps[:], lhsT=attn_big[:], rhs=pw_sb[:],
                             start=True, stop=True)
            p_sb = mid.tile([C, C], F32)
            nc.vector.tensor_copy(p_sb[:], p_ps[:])

            wf_ps = psum_mid.tile([C, C], F32, tag="mps")
            nc.tensor.matmul(wf_ps[:], lhsT=wvT_sb[:], rhs=p_sb[:],
                             start=True, stop=True)
            wf_bf = mid.tile([C, C], BF16)
            nc.vector.tensor_copy(wf_bf[:], wf_ps[:])

            bf_ps = psum_mid.tile([1, C], F32, tag="mps")
            nc.tensor.matmul(bf_ps[:], lhsT=vb_col[:], rhs=p_sb[:],
                             start=True, stop=True)
            bfin_row = mid.tile([1, C], F32)
            nc.vector.tensor_add(bfin_row[:], bf_ps[:], pb_row[:])

            bb_ps = psum_mid.tile([C, C], F32, tag="mps")
            nc.tensor.matmul(bb_ps[:], lhsT=ones_row[:], rhs=bfin_row[:],
                             start=True, stop=True)
            b_bc = mid.tile([C, C], F32)
            nc.vector.tensor_copy(b_bc[:], bb_ps[:])

            # ---- pass 2: Y = X @ Wf + bf ---------------------------------
            mid_ctx.close()
            with (
                tc.tile_pool(name="yout", bufs=3) as yout_pool,
                tc.tile_pool(name="psum_y", bufs=4, space="PSUM") as psum_y,
            ):
                for ch in range(NCH):
                    yout = yout_pool.tile([128, CHUNK, C], F32)
                    for n in range(CHUNK):
                        g = ch * CHUNK + n
                        y_ps = psum_y.tile([128, C], F32)
                        nc.tensor.matmul(y_ps[:],
                                         lhsT=xT_store[:, g * 128:(g + 1) * 128],
                                         rhs=wf_bf[:], start=True, stop=True)
                        nc.vector.tensor_add(yout[:, n, :], y_ps[:], b_bc[:])
                    nc.sync.dma_start(out_t[ch], yout[:])

    nc.compile()
    return nc


def kernel(x, qkv_w, q_bias, v_bias, scale, proj_w, proj_b, num_heads=4):
    global _CACHED_NC, LAST_EXEC_TIME_NS
    _install_ntff_hook()
    if _CACHED_NC is None:
        _CACHED_NC = build()
    nc = _CACHED_NC

    x = np.asarray(x, dtype=np.float32)
    qkv_w = np.asarray(qkv_w, dtype=np.float32)
    q_bias = np.asarray(q_bias, dtype=np.float32)
    v_bias = np.asarray(v_bias, dtype=np.float32)
    # reference reshapes qkv to (..., heads, 3, hd): column (h, t, d) of qkv_w
    # is h*96 + t*32 + d, and bias384 = concat(q_bias, 0, v_bias) is applied in
    # that same interleaved order. Permute host-side to [Wq | Wk | Wv] blocks
    # with matching effective biases (k picks up a nonzero bias).
    idx = np.concatenate([np.arange(h * 3 * HD, h * 3 * HD + HD)
                          for h in range(HEADS)])
    bias384 = np.concatenate([q_bias, np.zeros_like(q_bias), v_bias])
    w_perm = np.concatenate(
        [qkv_w[:, idx], qkv_w[:, idx + HD], qkv_w[:, idx + 2 * HD]], axis=1)
    shared = {
        "qkv_w": np.ascontiguousarray(w_perm),
        "q_bias": np.ascontiguousarray(bias384[idx]),
        "k_bias": np.ascontiguousarray(bias384[idx + HD]),
        "v_bias": np.ascontiguousarray(bias384[idx + 2 * HD]),
        "scale": np.ascontiguousarray(
            np.asarray(scale, dtype=np.float32).reshape(HEADS)),
        "proj_w": np.ascontiguousarray(np.asarray(proj_w, dtype=np.float32)),
        "proj_b": np.ascontiguousarray(np.asarray(proj_b, dtype=np.float32)),
    }
    in_maps = [
        {"x": np.ascontiguousarray(x[i].reshape(NTOK, C)), **shared}
        for i in range(B)
    ]
    trace = bool(os.environ.get("BASS_TRACE"))
    res = run_bass_kernel_spmd(nc, in_maps, core_ids=list(range(B)), trace=trace)
    LAST_EXEC_TIME_NS = res.exec_time_ns
    return np.stack([res.results[i]["out"].reshape(H, W, C) for i in range(B)])
